# revision 4
# baseline (speedup 1.0000x reference)
"""Half-Hadamard (64x64 block-diagonal channel transform) Trainium2 kernel.

Problem: x [8, 4096, 2048] f32, H [64, 64] f32 (scaled Hadamard).
    y[b, 64g+j, l] = sum_i x[b, 64g+i, l] * H[i, j]

Sharding: data-parallel over batch — core b handles x[b] ([4096, 2048]).

Per-core kernel: for each 128-channel group, y_grp = W^T @ x_grp where
W = blockdiag(H, H) [128, 128] is the stationary matmul operand
(out[j, l] = sum_i W[i, j] x[i, l]  ==  lhsT.T @ rhs with lhsT = W).
"""

import numpy as np

import concourse.bass as bass
import concourse.mybir as mybir
from concourse.tile import TileContext
from concourse.bass_utils import run_bass_kernel_spmd

B, C, L = 8, 4096, 2048
P = 128                # SBUF partitions = channels per matmul group
GPT = 2                # channel groups per DMA tile (tile = [P, GPT, L])
NTILES = C // (P * GPT)  # 16
NSPLIT = 512           # matmul moving free dim (fp32 max, one PSUM bank)
N_CORES = 8

_CACHE = {}


def _split_waits(nc, limit=1):
    """walrus codegen in this container accepts only ONE sync-wait per
    instruction; Tile emits up to ~3 (e.g. the kernel-tail drain). Hoist
    excess waits onto chained same-engine NoOps placed just before."""
    n_new = 0
    for f in nc.m.functions:
        for bb in f.blocks:
            new = []
            for inst in bb.instructions:
                si = inst.sync_info
                waits = list(si.on_wait) if (si and si.on_wait) else []
                if len(waits) > limit:
                    excess, keep = waits[:-limit], waits[-limit:]
                    for i in range(0, len(excess), limit):
                        chunk = excess[i:i + limit]
                        nop = mybir.InstNoOp(
                            name=f"waitsplit_{n_new}",
                            engine=inst.engine,
                            ins=[],
                            outs=[],
                            sync_info=mybir.SyncInfo(on_wait=chunk, on_update=[]),
                        )
                        n_new += 1
                        new.append(nop)
                    si.on_wait = keep
                new.append(inst)
            try:
                bb.instructions[:] = new
            except TypeError:
                bb.instructions = new
    return n_new


def build_bass(reps=1, split=True):
    """reps>1 repeats the whole pipeline in a hardware loop (timing only).
    split=False skips the walrus single-wait workaround (CoreSim's race
    detector can't execute the synthetic NoOps; walrus needs them)."""
    nc = bass.Bass("TRN2")
    x = nc.dram_tensor("x", (C, L), mybir.dt.float32, kind="ExternalInput")
    w = nc.dram_tensor("w", (P, P), mybir.dt.float32, kind="ExternalInput")
    y = nc.dram_tensor("y", (C, L), mybir.dt.float32, kind="ExternalOutput")

    xg = x.rearrange("(n t p) l -> n p t l", t=GPT, p=P)
    yg = y.rearrange("(n t p) l -> n p t l", t=GPT, p=P)

    with TileContext(nc) as tc:
        with (
            tc.tile_pool(name="const", bufs=1) as const_pool,
            tc.tile_pool(name="xin", bufs=3) as in_pool,
            tc.tile_pool(name="yout", bufs=3) as out_pool,
            tc.tile_pool(name="psum", bufs=8, space="PSUM") as psum_pool,
        ):
            wt = const_pool.tile([P, P], mybir.dt.float32)
            nc.sync.dma_start(out=wt[:], in_=w[:])

            def body(_i=None):
                for n in range(NTILES):
                    xt = in_pool.tile([P, GPT, L], mybir.dt.float32)
                    nc.sync.dma_start(out=xt[:], in_=xg[n])
                    ot = out_pool.tile([P, GPT, L], mybir.dt.float32)
                    for t in range(GPT):
                        for s in range(L // NSPLIT):
                            ps = psum_pool.tile([P, NSPLIT], mybir.dt.float32)
                            nc.tensor.matmul(
                                ps[:],
                                wt[:],
                                xt[:, t, bass.ts(s, NSPLIT)],
                                start=True,
                                stop=True,
                            )
                            # split PSUM->SBUF copies across DVE and ACT
                            eng = nc.vector if (t * 4 + s) % 2 == 0 else nc.scalar
                            if eng is nc.vector:
                                eng.tensor_copy(
                                    out=ot[:, t, bass.ts(s, NSPLIT)], in_=ps[:]
                                )
                            else:
                                eng.copy(ot[:, t, bass.ts(s, NSPLIT)], ps[:])
                    nc.sync.dma_start(out=yg[n], in_=ot[:])

            if reps == 1:
                body()
            else:
                with tc.For_i(0, reps, 1) as i:
                    body(i)
    if split:
        _split_waits(nc)
    return nc


def _weight(H: np.ndarray) -> np.ndarray:
    W = np.zeros((P, P), dtype=np.float32)
    W[:64, :64] = H
    W[64:, 64:] = H
    return W


def run(x, H, reps=1, **spmd_kwargs):
    """Full-input entry with passthrough kwargs for profiling/timing."""
    x = np.ascontiguousarray(np.asarray(x, dtype=np.float32))
    H = np.asarray(H, dtype=np.float32)
    assert x.shape == (B, C, L), x.shape
    W = _weight(H)
    key = ("nc", reps)
    if key not in _CACHE:
        _CACHE[key] = build_bass(reps)
    nc = _CACHE[key]
    in_maps = [{"x": x[i], "w": W} for i in range(N_CORES)]
    res = run_bass_kernel_spmd(nc, in_maps, core_ids=list(range(N_CORES)), **spmd_kwargs)
    out = np.stack([r["y"] for r in res.results], axis=0)
    return out, res


def kernel(x, H):
    out, _ = run(x, H)
    return out
